# revision 1
# baseline (speedup 1.0000x reference)
# 3x3/stride-1 VALID avg-pool over (16,256,64,64) f32, data-parallel over
# 8 NeuronCores (512 images/core, one image per SBUF partition), all four
# engine queues balanced (~26-29us each in the v1 cost model vs 47us of
# DMA on the baseline's single SP queue):
#   SP   : most top loads + b2/b3 bottom loads (fills its mid-run idle
#          window) + most stores
#   Act  : a 4-row lead DMA (a tiny first transfer pulls the queue's
#          first issue ~1.2us earlier), b0/b1 bottom loads, and all 1/9
#          scales (in-place activation mul, per row-chunk so stores
#          fire early)
#   Pool : horizontal 3-sums (mid) rows [0,PM), contiguous 2-add chains
#          in row chunks (GPSIMD-safe APs), plus relief stores
#   DVE  : mid rows [PM,64) + all vertical 3-sums via an even/odd stride
#          trick (1.5 adds per element)
# Block 0 is chunked finely to prime the pipeline; block 3's vert/scale/
# store ladder is chunked fine and fanned across queues so the last
# store is small and starts early.
# (tensor_tensor_reduce would fuse the scale for free but does not survive
# neuronxcc codegen -- "ISA wrong length" -- so scales live on Act.)
import numpy as np

MID_BF16 = False

N_CORES = 8
N, C = 16, 256
H = W = 64
OH = OW = 62
P = 128
IMGS_PER_CORE = (N * C) // N_CORES    # 512
NBLK = 4
RT = 34
PM = 58

_nc_cache = {}


def _split_multiwait(nc, max_waits=1):
    import concourse.mybir as mb

    for f in nc.m.functions:
        for b in f.blocks:
            new_list = []
            for inst in b.instructions:
                si = getattr(inst, "sync_info", None)
                if si is not None and len(si.on_wait) > max_waits:
                    waits = list(si.on_wait)
                    extra, keep = waits[:-max_waits], waits[-max_waits:]
                    for k, w in enumerate(extra):
                        es = mb.InstEventSemaphore(
                            name=f"{inst.name}-esw{k}", ins=[], outs=[],
                            engine=inst.engine)
                        es.sync_info = mb.SyncInfo(on_wait=[w], on_update=[])
                        nc.register_instruction(es)
                        new_list.append(es)
                    inst.sync_info = mb.SyncInfo(
                        on_wait=keep, on_update=list(si.on_update))
                new_list.append(inst)
            b.instructions[:] = new_list


def _build_nc():
    import concourse.bass as bass
    import concourse.mybir as mybir
    from concourse.tile import TileContext

    f32 = mybir.dt.float32
    fmid = mybir.dt.bfloat16 if MID_BF16 else f32

    nc = bass.Bass()
    x = nc.declare_dram_parameter("x", [IMGS_PER_CORE, H, W], f32, isOutput=False)
    o = nc.declare_dram_parameter("o", [IMGS_PER_CORE, OH, OW], f32, isOutput=True)

    with TileContext(nc) as tc:
        with (
            tc.tile_pool(name="xp", bufs=4) as xp,
            tc.tile_pool(name="mgp", bufs=3) as mgp,
            tc.tile_pool(name="ttp", bufs=2) as ttp,
            tc.tile_pool(name="tvp", bufs=3) as tvp,
            tc.tile_pool(name="op", bufs=3) as op,
            tc.tile_pool(name="wp", bufs=1) as wp,
        ):
            xbs = [xp.tile([P, H, W], f32, name="xb") for _ in range(NBLK)]

            def ld(eng, b, r0, r1):
                eng.dma_start(out=xbs[b][:, r0:r1, :],
                              in_=x[b * P:(b + 1) * P, r0:r1])

            # ---- loads ----
            # Act first: tiny top chunk of b0 (Act's queue issues earliest),
            # then b0's bottom tail (DVE trick-mid input), then the rest.
            ld(nc.scalar, 0, 0, 4)
            ld(nc.scalar, 0, 4, 17)
            ld(nc.scalar, 0, PM, H)
            ld(nc.scalar, 0, RT, PM)
            ld(nc.sync, 0, 17, RT)
            for b in range(1, NBLK):
                ld(nc.sync, b, 0, RT)
            for b in range(1, NBLK):
                ld(nc.scalar, b, PM, H)
            ld(nc.scalar, 1, RT, PM)
            ld(nc.sync, 2, RT, PM)
            ld(nc.sync, 3, RT, PM)

            mgs, ots = {}, {}

            def get_mg(b):
                if b not in mgs:
                    mgs[b] = mgp.tile([P, H, OW], fmid, name="mg")
                return mgs[b]

            def get_ot(b):
                if b not in ots:
                    ots[b] = op.tile([P, OH, OW], f32, name="ot")
                return ots[b]

            def pool_mid(b, r0, r1):
                xb, mg = xbs[b], get_mg(b)
                nc.gpsimd.tensor_add(
                    out=mg[:, r0:r1, :], in0=xb[:, r0:r1, 0:62],
                    in1=xb[:, r0:r1, 1:63])
                nc.gpsimd.tensor_add(
                    out=mg[:, r0:r1, :], in0=mg[:, r0:r1, :],
                    in1=xb[:, r0:r1, 2:64])

            def dve_trick_mid(b):
                # mid rows [PM,64) from x rows [PM,64)
                xb, mg = xbs[b], get_mg(b)
                nr = H - PM
                tt = ttp.tile([P, nr, 31], fmid, name="tt")
                nc.vector.tensor_add(
                    out=tt[:], in0=xb[:, PM:H, 1:62:2], in1=xb[:, PM:H, 2:63:2])
                nc.vector.tensor_add(
                    out=mg[:, PM:H, 0:61:2], in0=xb[:, PM:H, 0:61:2],
                    in1=tt[:, 0:nr, :])
                nc.vector.tensor_add(
                    out=mg[:, PM:H, 1:62:2], in0=tt[:, 0:nr, :],
                    in1=xb[:, PM:H, 3:64:2])

            def dve_vert(b, r0, r1):
                # out rows [r0,r1), r0/r1 even: needs mg rows r0..r1+1
                mg, ot = get_mg(b), get_ot(b)
                nh = (r1 - r0) // 2
                tv = tvp.tile([P, nh, OW], fmid, name="tv")
                nc.vector.tensor_add(
                    out=tv[:], in0=mg[:, r0 + 1:r1:2, :], in1=mg[:, r0 + 2:r1 + 1:2, :])
                nc.vector.tensor_add(
                    out=ot[:, r0:r1 - 1:2, :], in0=mg[:, r0:r1 - 1:2, :],
                    in1=tv[:, 0:nh, :])
                nc.vector.tensor_add(
                    out=ot[:, r0 + 1:r1:2, :], in0=tv[:, 0:nh, :],
                    in1=mg[:, r0 + 3:r1 + 2:2, :])

            def scale(b, r0, r1):
                ot = get_ot(b)
                nc.scalar.mul(out=ot[:, r0:r1, :], in_=ot[:, r0:r1, :],
                              mul=1.0 / 9.0)

            def store(b, r0, r1, eng):
                i0 = b * P
                ot = get_ot(b)
                eng.dma_start(out=o[i0:i0 + P, r0:r1, :], in_=ot[:, r0:r1, :])

            # ---- Pool chunk schedule ----
            # b0 finely chunked for the ramp; b3 split so the tail chain is
            # short; middles split A/B to keep DVE fed with bounded lag.
            pool_chunks = {
                0: [(0, 16), (16, RT), (RT, PM)],
                1: [(0, RT), (RT, PM)],
                2: [(0, RT), (RT, PM)],
                3: [(0, 32), (32, PM)],
            }
            # DVE per-block vert chunks (b0 split for the ramp)
            vert_chunks = {
                0: [(0, 14), (14, 30), (30, OH)],
                1: [(0, 26), (26, OH)],
                2: [(0, 26), (26, OH)],
                3: [(0, 16), (16, 30), (30, 46), (46, 54), (54, OH)],
            }

            # interleaved emission (per-queue program order):
            # store queue per (block, half): Pool relieves SP/Act midway
            st_q = {
                (0, 0): nc.sync, (0, 1): nc.sync,
                (1, 0): nc.sync, (1, 1): nc.gpsimd,
                (2, 0): nc.sync, (2, 1): nc.gpsimd,
                (3, 0): nc.sync, (3, 1): nc.sync,
            }
            # warm the Copy activation table during the load phase
            warm = wp.tile([P, 1], f32, name="warm")
            nc.vector.memset(warm[:], 0.0)
            nc.scalar.mul(out=warm[:], in_=warm[:], mul=1.0)

            dve_trick_mid(0)
            for b in range(NBLK):
                for (r0, r1) in pool_chunks[b]:
                    pool_mid(b, r0, r1)
                if b + 1 < NBLK:
                    dve_trick_mid(b + 1)
                for (r0, r1) in vert_chunks[b]:
                    dve_vert(b, r0, r1)
                    scale(b, r0, r1)
                if b < NBLK - 1:
                    bb = 30 if b == 0 else 26
                    store(b, 0, bb, st_q[(b, 0)])
                    store(b, bb, OH, st_q[(b, 1)])
                else:
                    store(b, 0, 16, nc.sync)
                    store(b, 16, 30, nc.sync)
                    store(b, 30, 46, nc.gpsimd)
                    store(b, 46, 54, nc.sync)
                    store(b, 54, OH, nc.scalar)

    _split_multiwait(nc)
    nc.finalize()
    return nc


def _get_nc():
    if "nc" not in _nc_cache:
        _nc_cache["nc"] = _build_nc()
    return _nc_cache["nc"]


def run(x, trace=False, **spmd_kwargs):
    from concourse.bass_utils import run_bass_kernel_spmd

    x = np.ascontiguousarray(np.asarray(x, dtype=np.float32))
    assert x.shape == (N, C, H, W), x.shape
    shards = x.reshape(N_CORES, IMGS_PER_CORE, H, W)
    in_maps = [{"x": shards[c]} for c in range(N_CORES)]
    nc = _get_nc()
    res = run_bass_kernel_spmd(
        nc, in_maps, list(range(N_CORES)), trace=trace, **spmd_kwargs
    )
    out = np.stack([res.results[c]["o"] for c in range(N_CORES)], axis=0)
    return out.reshape(N, C, OH, OW), res


def kernel(x):
    out, _ = run(x, trace=False)
    return out

